# revision 1
# baseline (speedup 1.0000x reference)
"""M2BertAttention Trainium2 Bass kernel.

B=1, S=4096, HID=768, NH=12 heads, HD=64. 8 NeuronCores.

Sharding: 8 cores = 4 head-groups (3 heads) x 2 query-halves (2048 q).
K/V projections duplicated across the 2 query-halves; no collectives.

Per-core layout (transposed attention):
  - host passes hs.T, packed/transposed weight slices, rope tables, mask
  - kT/qT projections: psum[128,512] = P.T @ hsT-tile, rope applied during
    PSUM->SBUF evacuation on DVE (scalar_tensor_tensor fuses bias add + cos/sin mul)
  - V in natural [s, d] layout with a ones column (denominator trick)
  - scoresT[sk,sq] = kT-chunk.T @ qT  (K=64)
  - probsT = exp(scoresT + mask) on ACT, mask folded into per-partition bias
  - ctxT[65,sq] += V-chunk.T @ probsT  (row 64 = softmax denominator)
  - normalize: reciprocal + partition-broadcast + multiply
"""

import sys

import numpy as np

try:
    import concourse.bass as bass
except ImportError:  # pragma: no cover
    sys.path.insert(0, "/opt/trn_rl_repo")
    import concourse.bass as bass

import concourse.mybir as mybir
import concourse.tile as tile
from concourse import bacc
from concourse.bass_utils import run_bass_kernel_spmd

S = 4096
HID = 768
NH = 12
HD = 64
HD2 = 32
HG = 3          # heads per core
SQ = S // 2     # queries per core
NCHUNK = S // 128   # 32 key chunks
NST = S // 512      # 8 seq tiles
F32 = mybir.dt.float32

# matmul operand dtype: float32r = full-rate fp32 path on the PE array
# (fp32 runs at 1/4 rate). All tensors feeding a matmul use this dtype.
MDT = mybir.dt.float32r


def _r(ap):
    return ap


def _build_kernel():
    nc = bacc.Bacc(None, target_bir_lowering=False)

    # pre-tiled on the host so every DMA is long-contiguous per partition
    hst8 = nc.dram_tensor("hst8", [NST, 128, 6, 512], MDT, kind="ExternalInput")
    p1 = nc.dram_tensor("p1", [128, 6, 128], MDT, kind="ExternalInput")
    p2 = nc.dram_tensor("p2", [128, 6, 128], MDT, kind="ExternalInput")
    p3 = nc.dram_tensor("p3", [128, 6, 128], MDT, kind="ExternalInput")
    wv = nc.dram_tensor("wv", [128, 6, 256], MDT, kind="ExternalInput")
    bcat = nc.dram_tensor("bcat", [128, 3], F32, kind="ExternalInput")
    c2k = nc.dram_tensor("c2k", [64, S], F32, kind="ExternalInput")
    s2k = nc.dram_tensor("s2k", [64, S], F32, kind="ExternalInput")
    maskc = nc.dram_tensor("maskc", [128, NCHUNK], F32, kind="ExternalInput")
    vones = nc.dram_tensor("vones", [128, 3 * NCHUNK], MDT, kind="ExternalInput")
    rowc = nc.dram_tensor("rowc", [1, 384], MDT, kind="ExternalInput")
    out = nc.dram_tensor("out", [HG, 64, SQ], F32, kind="ExternalOutput")

    ADD = mybir.AluOpType.add
    MULT = mybir.AluOpType.mult

    with tile.TileContext(nc) as tc:
        with (
            tc.tile_pool(name="persist", bufs=1) as persist,
            tc.tile_pool(name="small", bufs=1) as small,
        ):
            # persistent per-head tensors
            kts = [persist.tile([64, S], MDT, name=f"kt{h}", tag=f"kt{h}") for h in range(HG)]
            qts = [persist.tile([64, SQ], MDT, name=f"qt{h}", tag=f"qt{h}") for h in range(HG)]
            vt = persist.tile([128, NCHUNK, HG, 65], MDT, name="vt", tag="vt")
            masks = small.tile([128, NCHUNK], F32)
            rc = small.tile([1, 384], MDT)
            scr1 = small.tile([1, 1], F32)
            nc.sync.dma_start(out=rc, in_=rowc[:, :])
            onest = rc[0:1, 0:128]
            bvrt = rc[0:1, 128:384]
            # dummy exp: pulls the ACT exp table load off the critical path
            nc.scalar.activation(scr1, onest[0:1, 0:1], mybir.ActivationFunctionType.Exp)

            IDEN = mybir.ActivationFunctionType.Identity
            SUB = mybir.AluOpType.subtract
            stt = nc.vector.scalar_tensor_tensor

            # ---------------- projection phase ----------------
            with (
                tc.tile_pool(name="wpool", bufs=1) as wpool,
                tc.tile_pool(name="tabs", bufs=1) as tabs,
                tc.tile_pool(name="hst", bufs=2) as hstp,
                tc.tile_pool(name="pskq", bufs=3, space="PSUM") as pskq,
                tc.tile_pool(name="psv", bufs=2, space="PSUM") as psvp,
                tc.tile_pool(name="prer", bufs=4) as prer,
                tc.tile_pool(name="ropetmp", bufs=2) as rtmp,
            ):
                p1s = wpool.tile([128, 6, 128], MDT)
                p2s = wpool.tile([128, 6, 128], MDT)
                p3s = wpool.tile([128, 6, 128], MDT)
                wvs = wpool.tile([128, 6, 256], MDT)
                blo = wpool.tile([64, 3], F32)
                bhi = wpool.tile([64, 3], F32)
                nc.scalar.dma_start(out=p1s, in_=p1[:, :, :])
                nc.sync.dma_start(out=blo, in_=bcat[0:64, :])
                nc.sync.dma_start(out=bhi, in_=bcat[64:128, :])
                b1lo, b2lo, b3lo = blo[:, 0:1], blo[:, 1:2], blo[:, 2:3]
                b1hi, b2hi, b3hi = bhi[:, 0:1], bhi[:, 1:2], bhi[:, 2:3]
                c2ks = tabs.tile([64, S], F32)
                s2ks = tabs.tile([64, S], F32)

                def evac(ps, blo, bhi, need_hi=True):
                    """ACT copies psum [128,512] -> two [64,512] SBUF tiles
                    (base partition 0) with per-partition bias add."""
                    preA = prer.tile([64, 512], F32, name="preA", tag="preA")
                    nc.scalar.activation(preA, ps[0:64, :], IDEN, bias=blo)
                    if not need_hi:
                        return preA, None
                    preB = prer.tile([64, 512], F32, name="preB", tag="preB")
                    nc.scalar.activation(preB, ps[64:128, :], IDEN, bias=bhi)
                    return preA, preB

                def rope_pair(preA, preB, cos, sin, dst0, dst1, sl, eng=None):
                    """preA=[h0x1 h1x1], preB=[h0x2 h1x2]. Multiplies on GpSimd
                    or DVE (balanced); combines on DVE."""
                    eng = eng or nc.gpsimd
                    m1 = rtmp.tile([64, 512], F32, name="m1", tag="m1")
                    m2 = rtmp.tile([64, 512], F32, name="m2", tag="m2")
                    eng.tensor_mul(m1, preA, cos)
                    eng.tensor_mul(m2, preB, sin)
                    stt(dst0[0:32, sl], m1[0:32, :], 0.0, m2[0:32, :], ADD, SUB)
                    stt(dst1[0:32, sl], m1[32:64, :], 0.0, m2[32:64, :], ADD, SUB)
                    m3 = rtmp.tile([64, 512], F32, name="m3", tag="m3")
                    m4 = rtmp.tile([64, 512], F32, name="m4", tag="m4")
                    eng.tensor_mul(m3, preA, sin)
                    eng.tensor_mul(m4, preB, cos)
                    stt(dst0[32:64, sl], m3[0:32, :], 0.0, m4[0:32, :], ADD, ADD)
                    stt(dst1[32:64, sl], m3[32:64, :], 0.0, m4[32:64, :], ADD, ADD)

                def rope_one(pre, cos, sin, dst, sl, eng=None):
                    """pre [64,512] = [x1; x2] of one head; tables are
                    row-duplicated so rows 32:64 match base partition 32."""
                    eng = eng or nc.gpsimd
                    m1 = rtmp.tile([32, 512], F32, name="n1", tag="m1")
                    m2 = rtmp.tile([32, 512], F32, name="n2", tag="m2")
                    eng.tensor_mul(m1, pre[0:32, :], cos[0:32, :])
                    eng.tensor_mul(m2, pre[32:64, :], sin[32:64, :])
                    stt(dst[0:32, sl], m1, 0.0, m2, ADD, SUB)
                    m3 = rtmp.tile([32, 512], F32, name="n3", tag="m3")
                    m4 = rtmp.tile([32, 512], F32, name="n4", tag="m4")
                    eng.tensor_mul(m3, pre[0:32, :], sin[0:32, :])
                    eng.tensor_mul(m4, pre[32:64, :], cos[32:64, :])
                    stt(dst[32:64, sl], m3, 0.0, m4, ADD, ADD)

                for st in range(NST):
                    sl = bass.ds(st * 512, 512)
                    hst = hstp.tile([128, 6, 512], MDT)
                    if st == 0:
                        # chunked so the first matmul starts after 1/6 of
                        # the transfer
                        for ch in range(6):
                            nc.sync.dma_start(out=hst[:, ch], in_=hst8[st, :, ch])
                    else:
                        nc.sync.dma_start(out=hst, in_=hst8[st])
                    if st == 0:
                        for t, d in ((p2s, p2), (p3s, p3)):
                            nc.scalar.dma_start(out=t, in_=d[:, :, :])
                        nc.scalar.dma_start(out=wvs, in_=wv[:, :, :])
                    if st == 2:
                        nc.scalar.dma_start(
                            out=vt[:, :, :, 64],
                            in_=vones.rearrange("p (c h) -> p c h", h=HG))
                        nc.scalar.dma_start(out=masks, in_=maskc[:, :])
                    # rope-table chunk for this st only, keeps the serial DMA
                    # stream free for the next hst tile
                    nc.scalar.dma_start(out=c2ks[:, sl], in_=c2k[:, sl])
                    nc.scalar.dma_start(out=s2ks[:, sl], in_=s2k[:, sl])
                    ck = c2ks[:, sl]
                    sk = s2ks[:, sl]
                    # k pair (h0, h1)
                    ps = pskq.tile([128, 512], F32, name="ps", tag="ps")
                    for ch in range(6):
                        nc.tensor.matmul(
                            ps, _r(p1s[:, ch, :]), _r(hst[:, ch, :]),
                            start=(ch == 0), stop=(ch == 5),
                        )
                    preA, preB = evac(ps, b1lo, b1hi)
                    rope_pair(preA, preB, ck, sk, kts[0], kts[1], sl)
                    # k2 | q2
                    ps2 = pskq.tile([128, 512], F32, name="ps2", tag="ps")
                    for ch in range(6):
                        nc.tensor.matmul(
                            ps2, _r(p2s[:, ch, :]), _r(hst[:, ch, :]),
                            start=(ch == 0), stop=(ch == 5),
                        )
                    preC, preD = evac(ps2, b2lo, b2hi, need_hi=(st < 4))
                    rope_one(preC, ck, sk, kts[2], sl)
                    if st < 4:
                        slq = bass.ds(st * 512, 512)
                        rope_one(preD, ck, sk, qts[2], slq, eng=nc.vector)
                        # q pair (h0, h1)
                        ps3 = pskq.tile([128, 512], F32, name="ps3", tag="ps")
                        for ch in range(6):
                            nc.tensor.matmul(
                                ps3, _r(p3s[:, ch, :]), _r(hst[:, ch, :]),
                                start=(ch == 0), stop=(ch == 5),
                            )
                        preA, preB = evac(ps3, b3lo, b3hi)
                        rope_pair(preA, preB, ck, sk, qts[0], qts[1], slq,
                                  eng=nc.vector)
                    # v projection; bias via K=1 matmul, evacuation on ACT
                    for sc in range(4):
                        psv = psvp.tile([128, 256], F32, name="psv", tag="psv")
                        for ch in range(6):
                            nc.tensor.matmul(
                                psv,
                                _r(hst[:, ch, sc * 128 : (sc + 1) * 128]),
                                _r(wvs[:, ch, :]),
                                start=(ch == 0), stop=False,
                            )
                        nc.tensor.matmul(psv, _r(onest), _r(bvrt), start=False, stop=True)
                        ci = st * 4 + sc
                        nc.scalar.copy(
                            vt[:, ci, :, 0:64],
                            psv[:, 0:192].rearrange("p (h d) -> p h d", h=HG),
                        )

            # ---------------- attention phase ----------------
            with (
                tc.tile_pool(name="scps", bufs=3, space="PSUM") as scps,
                tc.tile_pool(name="ctxps", bufs=1, space="PSUM") as ctxps,
                tc.tile_pool(name="probs", bufs=3) as probsp,
                tc.tile_pool(name="normp", bufs=2) as normp,
                tc.tile_pool(name="outp", bufs=2) as outp,
            ):
                for h in range(HG):
                    for u in range(2):
                        qsl0 = u * 1024
                        ctxp = ctxps.tile([65, 1024], F32, name="ctx", tag="ctx")

                        def flush(pend):
                            pt, c = pend
                            for j in range(2):
                                nc.tensor.matmul(
                                    ctxp[:, j * 512 : (j + 1) * 512],
                                    _r(vt[:, c, h, :]),
                                    _r(pt[:, j * 512 : (j + 1) * 512]),
                                    start=(c == 0), stop=(c == NCHUNK - 1),
                                )

                        pend = None
                        for c in range(NCHUNK):
                            sp = scps.tile([128, 1024], F32, name="sp", tag="sp")
                            for j in range(2):
                                nc.tensor.matmul(
                                    sp[:, j * 512 : (j + 1) * 512],
                                    _r(kts[h][:, c * 128 : (c + 1) * 128]),
                                    _r(qts[h][:, qsl0 + j * 512 : qsl0 + (j + 1) * 512]),
                                    start=True, stop=True,
                                )
                            pt = probsp.tile([128, 1024], MDT, name="pt", tag="pt")
                            nc.scalar.activation(
                                pt, sp, mybir.ActivationFunctionType.Exp,
                                bias=masks[:, c : c + 1],
                            )
                            if pend is not None:
                                flush(pend)
                            pend = (pt, c)
                        flush(pend)
                        # normalize: one DVE copy releases the ctx psum tile
                        # fast; reciprocal runs off the SBUF copy; the
                        # partition-broadcast is a K=1 matmul
                        cs = normp.tile([65, 1024], F32, name="cs", tag="cs")
                        nc.vector.tensor_copy(cs, ctxp)
                        den = normp.tile([1, 1024], MDT, name="den", tag="den")
                        with nc.allow_low_precision(reason="f32r is 4-byte fp32"):
                            nc.vector.reciprocal(den, cs[64:65, :])
                        bc = ctxps.tile([65, 1024], F32, name="bc", tag="ctx")
                        for j in range(2):
                            nc.tensor.matmul(
                                bc[0:64, j * 512 : (j + 1) * 512],
                                _r(onest[:, 0:64]),
                                _r(den[:, j * 512 : (j + 1) * 512]),
                                start=True, stop=True,
                            )
                        ot = outp.tile([64, 1024], F32, name="ot", tag="ot")
                        nc.vector.tensor_mul(ot, cs[0:64, :], bc[0:64, :])
                        nc.sync.dma_start(
                            out=out[h][:, qsl0 : qsl0 + 1024], in_=ot)

    nc.compile()
    return nc


_NC_CACHE = None


def _get_nc():
    global _NC_CACHE
    if _NC_CACHE is None:
        _NC_CACHE = _build_kernel()
    return _NC_CACHE


def _rope_tables():
    """Bit-identical to the reference's f32 jax-on-cpu tables."""
    import jax
    import jax.numpy as jnp

    cpu = jax.devices("cpu")[0]
    with jax.default_device(cpu):
        inv_freq = 1.0 / (
            10000.0 ** (jnp.arange(0, HD, 2, dtype=jnp.float32) / HD)
        )
        t = jnp.arange(S, dtype=jnp.float32)
        freqs = t[:, None] * inv_freq[None, :]
        cos = np.asarray(jnp.cos(freqs), dtype=np.float32)
        sin = np.asarray(jnp.sin(freqs), dtype=np.float32)
    return cos, sin  # [S, HD2]


def _prep_inputs(hidden_states, attention_mask, Wq, bq, Wk, bk, Wv, bv):
    f = np.float32
    hs = np.asarray(hidden_states, dtype=f).reshape(S, HID)
    mask = np.asarray(attention_mask, dtype=f).reshape(S)
    Wq = np.asarray(Wq, dtype=f)
    Wk = np.asarray(Wk, dtype=f)
    Wv = np.asarray(Wv, dtype=f)
    bq = np.asarray(bq, dtype=f).reshape(HID)
    bk = np.asarray(bk, dtype=f).reshape(HID)
    bv = np.asarray(bv, dtype=f).reshape(HID)

    hsT = np.ascontiguousarray(hs.T)  # [HID, S]
    scale = f(1.0 / np.sqrt(HD).astype(f))
    WqT = np.ascontiguousarray(Wq.T) * scale  # fold 1/sqrt(d)
    bqs = bq * scale
    WkT = np.ascontiguousarray(Wk.T)
    WvT = np.ascontiguousarray(Wv.T)

    cos, sin = _rope_tables()
    cosT = np.ascontiguousarray(cos.T)  # [32, S]
    sinT = np.ascontiguousarray(sin.T)

    def packed_pair(WT, bvec, i0, i1):
        P = np.concatenate(
            [WT[:, i0 : i0 + 32], WT[:, i1 : i1 + 32],
             WT[:, i0 + 32 : i0 + 64], WT[:, i1 + 32 : i1 + 64]], axis=1)
        b = np.concatenate(
            [bvec[i0 : i0 + 32], bvec[i1 : i1 + 32],
             bvec[i0 + 32 : i0 + 64], bvec[i1 + 32 : i1 + 64]])
        return np.ascontiguousarray(P), np.ascontiguousarray(b.reshape(128, 1))

    in_maps = []
    for core in range(8):
        g, hf = core // 2, core % 2
        i0, i1, i2 = (3 * g) * 64, (3 * g + 1) * 64, (3 * g + 2) * 64
        qlo = hf * SQ
        perm = np.concatenate([np.arange(qlo, qlo + SQ), np.arange((1 - hf) * SQ, (1 - hf) * SQ + SQ)])

        P1, b1v = packed_pair(WkT, bk, i0, i1)
        P3, b3v = packed_pair(WqT, bqs, i0, i1)
        P2 = np.ascontiguousarray(
            np.concatenate([WkT[:, i2 : i2 + 64], WqT[:, i2 : i2 + 64]], axis=1))
        b2v = np.ascontiguousarray(
            np.concatenate([bk[i2 : i2 + 64], bqs[i2 : i2 + 64]]).reshape(128, 1))
        bcatv = np.ascontiguousarray(np.concatenate([b1v, b2v, b3v], axis=1))
        wvp = np.zeros((HID, 256), dtype=f)
        wvp[:, :192] = WvT[:, 3 * g * 64 : 3 * g * 64 + 192]
        bvr = np.zeros((1, 256), dtype=f)
        bvr[0, :192] = bv[3 * g * 64 : 3 * g * 64 + 192]
        rowcv = np.ascontiguousarray(
            np.concatenate([np.ones((1, 128), dtype=f), bvr], axis=1))

        cperm = cosT[:, perm]
        sperm = sinT[:, perm]
        c2kv = np.ascontiguousarray(np.concatenate([cperm, cperm], axis=0))
        s2kv = np.ascontiguousarray(np.concatenate([sperm, sperm], axis=0))
        maskv = np.ascontiguousarray(mask[perm].reshape(NCHUNK, 128).T)

        hst8 = np.ascontiguousarray(
            hsT[:, perm].reshape(6, 128, NST, 512).transpose(2, 1, 0, 3))

        def wtile(W):
            # [HID, M] -> [128, 6, M]
            return np.ascontiguousarray(W.reshape(6, 128, -1).transpose(1, 0, 2))

        in_maps.append({
            "hst8": hst8,
            "p1": wtile(P1), "p2": wtile(P2), "p3": wtile(P3), "wv": wtile(wvp),
            "bcat": bcatv,
            "c2k": c2kv, "s2k": s2kv, "maskc": maskv,
            "vones": np.ones((128, 3 * NCHUNK), dtype=f),
            "rowc": rowcv,
        })
    return in_maps


def _assemble(results):
    A = np.stack([results[c]["out"] for c in range(8)])  # [8, 3, 64, SQ]
    A = A.reshape(4, 2, HG, 64, SQ)          # [g, hf, j, d, qq]
    full = A.transpose(1, 4, 0, 2, 3).reshape(S, HID)  # [(hf qq), (g j d)]
    return np.ascontiguousarray(full.reshape(1, S, HID).astype(np.float32))


def kernel(hidden_states, attention_mask, Wq, bq, Wk, bk, Wv, bv, _trace=False):
    nc = _get_nc()
    in_maps = _prep_inputs(hidden_states, attention_mask, Wq, bq, Wk, bk, Wv, bv)
    res = run_bass_kernel_spmd(nc, in_maps, core_ids=list(range(8)), trace=_trace)
    out = _assemble(res.results)
    if _trace:
        return out, res
    return out


if __name__ == "__main__":
    rng = np.random.default_rng(0)
    ins = {
        "hidden_states": rng.standard_normal((1, S, HID), dtype=np.float32),
        "attention_mask": np.zeros((1, 1, 1, S), dtype=np.float32),
        "Wq": (rng.standard_normal((HID, HID)) * 0.02).astype(np.float32),
        "bq": np.zeros(HID, np.float32),
        "Wk": (rng.standard_normal((HID, HID)) * 0.02).astype(np.float32),
        "bk": np.zeros(HID, np.float32),
        "Wv": (rng.standard_normal((HID, HID)) * 0.02).astype(np.float32),
        "bv": np.zeros(HID, np.float32),
    }
    out = kernel(**ins)
    print("kernel output", out.shape, out.dtype, np.abs(out).max())



# revision 6
# speedup vs baseline: 1.3879x; 1.3879x over previous
"""M2BertAttention Trainium2 Bass kernel.

B=1, S=4096, HID=768, NH=12 heads, HD=64. 8 NeuronCores.

Sharding: 8 cores = 4 head-groups (3 heads) x 2 query-halves (2048 q).
K/V projections duplicated across the 2 query-halves; no collectives.

Per-core pipeline:
  - projections in bf16 (weights + hidden states), rope applied on
    DVE/GpSimd out of PSUM evacuations, K/Q written twice (partitions
    0:64 and 64:128) so score matmuls can use 64x128 row tiling
  - scoresT[k,q]: two concurrent 64-row-tile matmuls per 128-key chunk
    (T0 covers q 0:512, T8 covers q 512:1024)
  - probs = exp(scores + mask): 2/3 of chunks on ACT (LUT exp with mask
    as per-partition bias), 1/3 on DVE via the Schraudolph bit trick
    (int32(A*s + B) bit-pattern ~= fp32 exp, ~3% sawtooth error that
    washes out over 4096 keys)
  - ctxT[65,q] += V-chunk.T @ probsT, row 64 = softmax denominator via a
    ones column in V
  - normalize: broadcast denominator via K=1 matmul, then one
    reciprocal_approx_fast + multiply
"""

import sys

import numpy as np

try:
    import concourse.bass as bass
except ImportError:  # pragma: no cover
    sys.path.insert(0, "/opt/trn_rl_repo")
    import concourse.bass as bass

import ml_dtypes
import concourse.mybir as mybir
import concourse.tile as tile
from concourse import bacc
from concourse.bass_utils import run_bass_kernel_spmd

S = 4096
HID = 768
NH = 12
HD = 64
HD2 = 32
HG = 3          # heads per core
SQ = S // 2     # queries per core
NCHUNK = S // 128   # 32 key chunks
NST = S // 512      # 8 seq tiles
F32 = mybir.dt.float32
BF = mybir.dt.bfloat16
MDT = mybir.dt.float32r
I16 = mybir.dt.int16

# Schraudolph fast-exp constants: bitcast(int32(A*x + B)) ~= exp(x).
# B is calibrated so the mean log-error over the mantissa sawtooth is zero.
SCHRA_A = 184.6650390625
SCHRA_B = 16248.7
# chunks with c % 3 == SCHRA_PHASE run exp on DVE instead of ACT
SCHRA_MOD = 3
SCHRA_PHASE = 2


def _build_kernel():
    nc = bacc.Bacc(None, target_bir_lowering=False)

    # pre-tiled on the host so every DMA is long-contiguous per partition
    hst8 = nc.dram_tensor("hst8", [NST, 128, 6, 512], BF, kind="ExternalInput")
    p1 = nc.dram_tensor("p1", [128, 6, 128], BF, kind="ExternalInput")
    p2 = nc.dram_tensor("p2", [128, 6, 128], BF, kind="ExternalInput")
    p3 = nc.dram_tensor("p3", [128, 6, 128], BF, kind="ExternalInput")
    wv = nc.dram_tensor("wv", [128, 6, 256], BF, kind="ExternalInput")
    bcat = nc.dram_tensor("bcat", [128, 3], F32, kind="ExternalInput")
    c2k = nc.dram_tensor("c2k", [64, S], BF, kind="ExternalInput")
    s2k = nc.dram_tensor("s2k", [64, S], BF, kind="ExternalInput")
    maskc = nc.dram_tensor("maskc", [128, NCHUNK], F32, kind="ExternalInput")
    maskab = nc.dram_tensor("maskab", [128, NCHUNK], F32, kind="ExternalInput")
    vones = nc.dram_tensor("vones", [128, 3 * NCHUNK], BF, kind="ExternalInput")
    rowv = nc.dram_tensor("rowv", [1, 384], BF, kind="ExternalInput")
    rowo = nc.dram_tensor("rowo", [1, 64], MDT, kind="ExternalInput")
    out = nc.dram_tensor("out", [HG, 64, SQ], F32, kind="ExternalOutput")

    ADD = mybir.AluOpType.add
    MULT = mybir.AluOpType.mult

    with tile.TileContext(nc) as tc:
        with (
            tc.tile_pool(name="persist", bufs=1) as persist,
            tc.tile_pool(name="small", bufs=1) as small,
        ):
            # persistent per-head tensors; partitions 64:128 of kt/qt hold a
            # duplicate of 0:64 for the 64x128 row-tiled score matmuls
            kts = [persist.tile([128, S], BF, name=f"kt{h}", tag=f"kt{h}") for h in range(HG)]
            qts = [persist.tile([128, SQ], BF, name=f"qt{h}", tag=f"qt{h}") for h in range(HG)]
            vt = persist.tile([128, NCHUNK, HG, 65], BF, name="vt", tag="vt")
            masks = small.tile([128, NCHUNK], F32)
            maskabs = small.tile([128, NCHUNK], F32)
            rv = small.tile([1, 384], BF)
            ro = small.tile([1, 64], MDT)
            scr1 = small.tile([1, 1], F32)
            nc.sync.dma_start(out=rv, in_=rowv[:, :])
            nc.sync.dma_start(out=ro, in_=rowo[:, :])
            onesv = rv[0:1, 0:128]
            bvrt = rv[0:1, 128:384]
            # dummy exp: pulls the ACT exp table load off the critical path
            nc.scalar.activation(scr1, ro[0:1, 0:1], mybir.ActivationFunctionType.Exp)

            IDEN = mybir.ActivationFunctionType.Identity
            SUB = mybir.AluOpType.subtract
            stt = nc.vector.scalar_tensor_tensor

            # ---------------- projection phase ----------------
            with (
                tc.tile_pool(name="wpool", bufs=1) as wpool,
                tc.tile_pool(name="tabs", bufs=1) as tabs,
                tc.tile_pool(name="hst", bufs=2) as hstp,
                tc.tile_pool(name="pskq", bufs=3, space="PSUM") as pskq,
                tc.tile_pool(name="psv", bufs=2, space="PSUM") as psvp,
                tc.tile_pool(name="prer", bufs=4) as prer,
                tc.tile_pool(name="ropetmp", bufs=2) as rtmp,
            ):
                p1s = wpool.tile([128, 6, 128], BF)
                p2s = wpool.tile([128, 6, 128], BF)
                p3s = wpool.tile([128, 6, 128], BF)
                wvs = wpool.tile([128, 6, 256], BF)
                blo = wpool.tile([64, 3], F32)
                bhi = wpool.tile([64, 3], F32)
                nc.scalar.dma_start(out=p1s, in_=p1[:, :, :])
                nc.sync.dma_start(out=blo, in_=bcat[0:64, :])
                nc.sync.dma_start(out=bhi, in_=bcat[64:128, :])
                b1lo, b2lo, b3lo = blo[:, 0:1], blo[:, 1:2], blo[:, 2:3]
                b1hi, b2hi, b3hi = bhi[:, 0:1], bhi[:, 1:2], bhi[:, 2:3]
                c2ks = tabs.tile([64, S], BF)
                s2ks = tabs.tile([64, S], BF)

                def evac(ps, blo, bhi, need_hi=True):
                    """ACT copies psum [128,512] -> two [64,512] bf16 SBUF
                    tiles (base partition 0) with per-partition bias add."""
                    preA = prer.tile([64, 512], BF, name="preA", tag="preA")
                    nc.scalar.activation(preA, ps[0:64, :], IDEN, bias=blo)
                    if not need_hi:
                        return preA, None
                    preB = prer.tile([64, 512], BF, name="preB", tag="preB")
                    nc.scalar.activation(preB, ps[64:128, :], IDEN, bias=bhi)
                    return preA, preB

                def rope_pair(preA, preB, cos, sin, dst0, dst1, sl, eng=None):
                    """preA=[h0x1 h1x1], preB=[h0x2 h1x2]. Multiplies on GpSimd
                    or DVE (balanced); combines on DVE."""
                    eng = eng or nc.gpsimd
                    m1 = rtmp.tile([64, 512], BF, name="m1", tag="m1")
                    m2 = rtmp.tile([64, 512], BF, name="m2", tag="m2")
                    eng.tensor_mul(m1, preA, cos)
                    eng.tensor_mul(m2, preB, sin)
                    stt(dst0[0:32, sl], m1[0:32, :], 0.0, m2[0:32, :], ADD, SUB)
                    stt(dst1[0:32, sl], m1[32:64, :], 0.0, m2[32:64, :], ADD, SUB)
                    m3 = rtmp.tile([64, 512], BF, name="m3", tag="m3")
                    m4 = rtmp.tile([64, 512], BF, name="m4", tag="m4")
                    eng.tensor_mul(m3, preA, sin)
                    eng.tensor_mul(m4, preB, cos)
                    stt(dst0[32:64, sl], m3[0:32, :], 0.0, m4[0:32, :], ADD, ADD)
                    stt(dst1[32:64, sl], m3[32:64, :], 0.0, m4[32:64, :], ADD, ADD)

                def rope_one(pre, cos, sin, dst, sl, eng=None):
                    """pre [64,512] = [x1; x2] of one head; tables are
                    row-duplicated so rows 32:64 match base partition 32."""
                    eng = eng or nc.gpsimd
                    m1 = rtmp.tile([32, 512], BF, name="n1", tag="m1")
                    m2 = rtmp.tile([32, 512], BF, name="n2", tag="m2")
                    eng.tensor_mul(m1, pre[0:32, :], cos[0:32, :])
                    eng.tensor_mul(m2, pre[32:64, :], sin[32:64, :])
                    stt(dst[0:32, sl], m1, 0.0, m2, ADD, SUB)
                    m3 = rtmp.tile([32, 512], BF, name="n3", tag="m3")
                    m4 = rtmp.tile([32, 512], BF, name="n4", tag="m4")
                    eng.tensor_mul(m3, pre[0:32, :], sin[0:32, :])
                    eng.tensor_mul(m4, pre[32:64, :], cos[32:64, :])
                    stt(dst[32:64, sl], m3, 0.0, m4, ADD, ADD)

                for st in range(NST):
                    sl = bass.ds(st * 512, 512)
                    hst = hstp.tile([128, 6, 512], BF)
                    if st == 0:
                        # chunked so the first matmul starts after 1/6 of
                        # the transfer
                        for ch in range(6):
                            nc.sync.dma_start(out=hst[:, ch], in_=hst8[st, :, ch])
                    else:
                        nc.sync.dma_start(out=hst, in_=hst8[st])
                    if st == 0:
                        for t, d in ((p2s, p2), (p3s, p3)):
                            nc.scalar.dma_start(out=t, in_=d[:, :, :])
                        nc.scalar.dma_start(out=wvs, in_=wv[:, :, :])
                    if st == 2:
                        nc.scalar.dma_start(
                            out=vt[:, :, :, 64],
                            in_=vones.rearrange("p (c h) -> p c h", h=HG))
                        nc.scalar.dma_start(out=masks, in_=maskc[:, :])
                        nc.scalar.dma_start(out=maskabs, in_=maskab[:, :])
                    # rope-table chunk for this st only, keeps the serial DMA
                    # stream free for the next hst tile
                    nc.scalar.dma_start(out=c2ks[:, sl], in_=c2k[:, sl])
                    nc.scalar.dma_start(out=s2ks[:, sl], in_=s2k[:, sl])
                    ck = c2ks[:, sl]
                    sk = s2ks[:, sl]
                    # k pair (h0, h1)
                    ps = pskq.tile([128, 512], F32, name="ps", tag="ps")
                    for ch in range(6):
                        nc.tensor.matmul(
                            ps, p1s[:, ch, :], hst[:, ch, :],
                            start=(ch == 0), stop=(ch == 5),
                        )
                    preA, preB = evac(ps, b1lo, b1hi)
                    rope_pair(preA, preB, ck, sk, kts[0], kts[1], sl)
                    # k2 | q2
                    ps2 = pskq.tile([128, 512], F32, name="ps2", tag="ps")
                    for ch in range(6):
                        nc.tensor.matmul(
                            ps2, p2s[:, ch, :], hst[:, ch, :],
                            start=(ch == 0), stop=(ch == 5),
                        )
                    preC, preD = evac(ps2, b2lo, b2hi, need_hi=(st < 4))
                    rope_one(preC, ck, sk, kts[2], sl)
                    if st < 4:
                        slq = bass.ds(st * 512, 512)
                        rope_one(preD, ck, sk, qts[2], slq, eng=nc.vector)
                        # q pair (h0, h1)
                        ps3 = pskq.tile([128, 512], F32, name="ps3", tag="ps")
                        for ch in range(6):
                            nc.tensor.matmul(
                                ps3, p3s[:, ch, :], hst[:, ch, :],
                                start=(ch == 0), stop=(ch == 5),
                            )
                        preA, preB = evac(ps3, b3lo, b3hi)
                        rope_pair(preA, preB, ck, sk, qts[0], qts[1], slq,
                                  eng=nc.vector)
                        # duplicate q rows into partitions 64:128 for the
                        # T8 score tiles
                        for h in range(HG):
                            nc.vector.tensor_copy(
                                qts[h][64:128, slq], qts[h][0:64, slq])
                    # duplicate k rows into partitions 64:128
                    for h in range(HG):
                        nc.gpsimd.tensor_copy(
                            kts[h][64:128, sl], kts[h][0:64, sl])
                    # v projection; bias via K=1 matmul, evacuation on ACT
                    for sc in range(4):
                        psv = psvp.tile([128, 256], F32, name="psv", tag="psv")
                        for ch in range(6):
                            nc.tensor.matmul(
                                psv,
                                hst[:, ch, sc * 128 : (sc + 1) * 128],
                                wvs[:, ch, :],
                                start=(ch == 0), stop=False,
                            )
                        nc.tensor.matmul(psv, onesv, bvrt, start=False, stop=True)
                        ci = st * 4 + sc
                        nc.scalar.copy(
                            vt[:, ci, :, 0:64],
                            psv[:, 0:192].rearrange("p (h d) -> p h d", h=HG),
                        )

            # ---------------- attention phase ----------------
            with (
                tc.tile_pool(name="scps", bufs=3, space="PSUM") as scps,
                tc.tile_pool(name="ctxps", bufs=1, space="PSUM") as ctxps,
                tc.tile_pool(name="probs", bufs=4) as probsp,
                tc.tile_pool(name="normp", bufs=2) as normp,
                tc.tile_pool(name="outp", bufs=2) as outp,
            ):
                for h in range(HG):
                    for u in range(2):
                        qsl0 = u * 1024
                        ctxp = ctxps.tile([65, 1024], F32, name="ctx", tag="ctx")

                        def flush(pend):
                            for pt, c in pend:
                                for j in range(2):
                                    nc.tensor.matmul(
                                        ctxp[:, j * 512 : (j + 1) * 512],
                                        vt[:, c, h, :],
                                        pt[:, j * 512 : (j + 1) * 512],
                                        start=(c == 0), stop=(c == NCHUNK - 1),
                                    )

                        pend = None
                        for p in range(NCHUNK // 2):
                            cur = []
                            for c in (2 * p, 2 * p + 1):
                                c0 = c * 128
                                sp = scps.tile([128, 1024], F32, name="sp", tag="sp")
                                nc.tensor.matmul(
                                    sp[:, 0:512],
                                    kts[h][0:64, c0 : c0 + 128],
                                    qts[h][0:64, qsl0 : qsl0 + 512],
                                    start=True, stop=True,
                                )
                                nc.tensor.matmul(
                                    sp[:, 512:1024],
                                    kts[h][64:128, c0 : c0 + 128],
                                    qts[h][64:128, qsl0 + 512 : qsl0 + 1024],
                                    start=True, stop=True,
                                )
                                pt = probsp.tile([128, 1024], BF, name="pt", tag="pt")
                                if c % SCHRA_MOD == SCHRA_PHASE:
                                    # fast exp: bit pattern of A*s + B
                                    nc.vector.tensor_scalar(
                                        pt[:, :].bitcast(I16), sp,
                                        SCHRA_A, maskabs[:, c : c + 1],
                                        MULT, ADD,
                                    )
                                else:
                                    nc.scalar.activation(
                                        pt, sp, mybir.ActivationFunctionType.Exp,
                                        bias=masks[:, c : c + 1],
                                    )
                                cur.append((pt, c))
                            if pend is not None:
                                flush(pend)
                            pend = cur
                        flush(pend)
                        # normalize: broadcast the denominator row via a K=1
                        # matmul, then approx-reciprocal + multiply
                        cs = normp.tile([64, 1024], MDT, name="cs", tag="cs")
                        nc.vector.tensor_copy(cs, ctxp[0:64, :])
                        csd = normp.tile([1, 1024], MDT, name="csd", tag="csd")
                        nc.scalar.copy(csd, ctxp[64:65, :])
                        bc = ctxps.tile([64, 1024], F32, name="bc", tag="ctx")
                        for j in range(2):
                            nc.tensor.matmul(
                                bc[:, j * 512 : (j + 1) * 512],
                                ro[:, :],
                                csd[:, j * 512 : (j + 1) * 512],
                                start=True, stop=True,
                            )
                        rb = normp.tile([64, 1024], F32, name="rb", tag="rb")
                        nc.vector.reciprocal_approx_fast(rb[:, :], bc[:, :])
                        ot = outp.tile([64, 1024], F32, name="ot", tag="ot")
                        nc.gpsimd.tensor_mul(
                            ot, cs[:, :].bitcast(F32), rb)
                        nc.sync.dma_start(
                            out=out[h][:, qsl0 : qsl0 + 1024], in_=ot)

    nc.compile()
    return nc


_NC_CACHE = None


def _get_nc():
    global _NC_CACHE
    if _NC_CACHE is None:
        _NC_CACHE = _build_kernel()
    return _NC_CACHE


def _rope_tables():
    """Bit-identical to the reference's f32 jax-on-cpu tables."""
    import jax
    import jax.numpy as jnp

    cpu = jax.devices("cpu")[0]
    with jax.default_device(cpu):
        inv_freq = 1.0 / (
            10000.0 ** (jnp.arange(0, HD, 2, dtype=jnp.float32) / HD)
        )
        t = jnp.arange(S, dtype=jnp.float32)
        freqs = t[:, None] * inv_freq[None, :]
        cos = np.asarray(jnp.cos(freqs), dtype=np.float32)
        sin = np.asarray(jnp.sin(freqs), dtype=np.float32)
    return cos, sin  # [S, HD2]


def _prep_inputs(hidden_states, attention_mask, Wq, bq, Wk, bk, Wv, bv):
    f = np.float32
    bf = ml_dtypes.bfloat16
    hs = np.asarray(hidden_states, dtype=f).reshape(S, HID)
    mask = np.asarray(attention_mask, dtype=f).reshape(S)
    Wq = np.asarray(Wq, dtype=f)
    Wk = np.asarray(Wk, dtype=f)
    Wv = np.asarray(Wv, dtype=f)
    bq = np.asarray(bq, dtype=f).reshape(HID)
    bk = np.asarray(bk, dtype=f).reshape(HID)
    bv = np.asarray(bv, dtype=f).reshape(HID)

    hsTb = np.ascontiguousarray(hs.T).astype(bf)  # [HID, S]
    scale = f(1.0 / np.sqrt(HD).astype(f))
    WqT = np.ascontiguousarray(Wq.T) * scale  # fold 1/sqrt(d)
    bqs = bq * scale
    WkT = np.ascontiguousarray(Wk.T)
    WvT = np.ascontiguousarray(Wv.T)

    cos, sin = _rope_tables()
    cosT = np.ascontiguousarray(cos.T)  # [32, S]
    sinT = np.ascontiguousarray(sin.T)

    def packed_pair(WT, bvec, i0, i1):
        P = np.concatenate(
            [WT[:, i0 : i0 + 32], WT[:, i1 : i1 + 32],
             WT[:, i0 + 32 : i0 + 64], WT[:, i1 + 32 : i1 + 64]], axis=1)
        b = np.concatenate(
            [bvec[i0 : i0 + 32], bvec[i1 : i1 + 32],
             bvec[i0 + 32 : i0 + 64], bvec[i1 + 32 : i1 + 64]])
        return np.ascontiguousarray(P), np.ascontiguousarray(b.reshape(128, 1))

    def wtile(W):
        # [HID, M] -> [128, 6, M], bf16
        return np.ascontiguousarray(
            W.reshape(6, 128, -1).transpose(1, 0, 2).astype(bf))

    # per-query-half shared tensors (only 2 variants, reused by 4 cores each)
    half = {}
    for hf in range(2):
        perm = np.concatenate([
            np.arange(hf * SQ, hf * SQ + SQ),
            np.arange((1 - hf) * SQ, (1 - hf) * SQ + SQ)])
        cperm = cosT[:, perm]
        sperm = sinT[:, perm]
        c2kv = np.ascontiguousarray(
            np.concatenate([cperm, cperm], axis=0).astype(bf))
        s2kv = np.ascontiguousarray(
            np.concatenate([sperm, sperm], axis=0).astype(bf))
        mperm = mask[perm]
        maskv = np.ascontiguousarray(mperm.reshape(NCHUNK, 128).T)
        maskabv = np.ascontiguousarray(
            (mperm * np.float32(SCHRA_A) + np.float32(SCHRA_B))
            .astype(f).reshape(NCHUNK, 128).T)
        hst8 = np.ascontiguousarray(
            hsTb[:, perm].reshape(6, 128, NST, 512).transpose(2, 1, 0, 3))
        half[hf] = (hst8, c2kv, s2kv, maskv, maskabv)

    in_maps = []
    for core in range(8):
        g, hf = core // 2, core % 2
        i0, i1 = (3 * g) * 64, (3 * g + 1) * 64
        i2 = (3 * g + 2) * 64

        P1, _b1 = packed_pair(WkT, bk, i0, i1)
        P3, _b3 = packed_pair(WqT, bqs, i0, i1)
        b1v = _b1
        b3v = _b3
        P2 = np.ascontiguousarray(
            np.concatenate([WkT[:, i2 : i2 + 64], WqT[:, i2 : i2 + 64]], axis=1))
        b2v = np.ascontiguousarray(
            np.concatenate([bk[i2 : i2 + 64], bqs[i2 : i2 + 64]]).reshape(128, 1))
        bcatv = np.ascontiguousarray(np.concatenate([b1v, b2v, b3v], axis=1))
        wvp = np.zeros((HID, 256), dtype=f)
        wvp[:, :192] = WvT[:, 3 * g * 64 : 3 * g * 64 + 192]
        bvr = np.zeros((1, 256), dtype=f)
        bvr[0, :192] = bv[3 * g * 64 : 3 * g * 64 + 192]
        rowvv = np.ascontiguousarray(
            np.concatenate([np.ones((1, 128), dtype=f), bvr], axis=1).astype(bf))

        hst8, c2kv, s2kv, maskv, maskabv = half[hf]
        in_maps.append({
            "hst8": hst8,
            "p1": wtile(P1), "p2": wtile(P2), "p3": wtile(P3),
            "wv": wtile(wvp),
            "bcat": bcatv,
            "c2k": c2kv, "s2k": s2kv,
            "maskc": maskv, "maskab": maskabv,
            "vones": np.ones((128, 3 * NCHUNK), dtype=ml_dtypes.bfloat16),
            "rowv": rowvv,
            "rowo": np.ones((1, 64), dtype=f),
        })
    return in_maps


def _assemble(results):
    A = np.stack([results[c]["out"] for c in range(8)])  # [8, 3, 64, SQ]
    A = A.reshape(4, 2, HG, 64, SQ)          # [g, hf, j, d, qq]
    full = A.transpose(1, 4, 0, 2, 3).reshape(S, HID)  # [(hf qq), (g j d)]
    return np.ascontiguousarray(full.reshape(1, S, HID).astype(np.float32))


def kernel(hidden_states, attention_mask, Wq, bq, Wk, bk, Wv, bv, _trace=False):
    nc = _get_nc()
    in_maps = _prep_inputs(hidden_states, attention_mask, Wq, bq, Wk, bk, Wv, bv)
    res = run_bass_kernel_spmd(nc, in_maps, core_ids=list(range(8)), trace=_trace)
    out = _assemble(res.results)
    if _trace:
        return out, res
    return out


if __name__ == "__main__":
    rng = np.random.default_rng(0)
    ins = {
        "hidden_states": rng.standard_normal((1, S, HID), dtype=np.float32),
        "attention_mask": np.zeros((1, 1, 1, S), dtype=np.float32),
        "Wq": (rng.standard_normal((HID, HID)) * 0.02).astype(np.float32),
        "bq": np.zeros(HID, np.float32),
        "Wk": (rng.standard_normal((HID, HID)) * 0.02).astype(np.float32),
        "bk": np.zeros(HID, np.float32),
        "Wv": (rng.standard_normal((HID, HID)) * 0.02).astype(np.float32),
        "bv": np.zeros(HID, np.float32),
    }
    out = kernel(**ins)
    print("kernel output", out.shape, out.dtype, np.abs(out).max())


# revision 8
# speedup vs baseline: 1.4904x; 1.0739x over previous
"""M2BertAttention Trainium2 Bass kernel.

B=1, S=4096, HID=768, NH=12 heads, HD=64. 8 NeuronCores.

Sharding: 8 cores = 4 head-groups (3 heads) x 2 query-halves (2048 q).
K/V projections duplicated across the 2 query-halves; no collectives.

Per-core pipeline:
  - projections in bf16 (weights + hidden states), rope applied on
    DVE/GpSimd out of PSUM evacuations, K/Q written twice (partitions
    0:64 and 64:128) so score matmuls can use 64x128 row tiling
  - scoresT[k,q]: two concurrent 64-row-tile matmuls per 128-key chunk
    (T0 covers q 0:512, T8 covers q 512:1024)
  - probs = exp(scores + mask): 2/3 of chunks on ACT (LUT exp with mask
    as per-partition bias), 1/3 on DVE via the Schraudolph bit trick
    (int32(A*s + B) bit-pattern ~= fp32 exp, ~3% sawtooth error that
    washes out over 4096 keys)
  - ctxT[65,q] += V-chunk.T @ probsT, row 64 = softmax denominator via a
    ones column in V
  - normalize: broadcast denominator via K=1 matmul, then one
    reciprocal_approx_fast + multiply
"""

import sys

import numpy as np

try:
    import concourse.bass as bass
except ImportError:  # pragma: no cover
    sys.path.insert(0, "/opt/trn_rl_repo")
    import concourse.bass as bass

import ml_dtypes
import concourse.mybir as mybir
import concourse.tile as tile
from concourse import bacc
from concourse.bass_utils import run_bass_kernel_spmd

S = 4096
HID = 768
NH = 12
HD = 64
HD2 = 32
HG = 3          # heads per core
SQ = S // 2     # queries per core
NCHUNK = S // 128   # 32 key chunks
NST = S // 512      # 8 seq tiles
F32 = mybir.dt.float32
BF = mybir.dt.bfloat16
MDT = mybir.dt.float32r
I16 = mybir.dt.int16

# Schraudolph fast-exp constants: bitcast(int32(A*x + B)) ~= exp(x).
# B is calibrated so the mean log-error over the mantissa sawtooth is zero.
SCHRA_A = 184.6650390625
SCHRA_B = 16248.7
# chunks with c % 3 == SCHRA_PHASE run exp on DVE instead of ACT
SCHRA_MOD = 3
SCHRA_PHASE = 2


def _build_kernel():
    nc = bacc.Bacc(None, target_bir_lowering=False)

    # pre-tiled on the host so every DMA is long-contiguous per partition
    hst8 = nc.dram_tensor("hst8", [NST, 128, 6, 512], BF, kind="ExternalInput")
    p1 = nc.dram_tensor("p1", [128, 6, 128], BF, kind="ExternalInput")
    p2 = nc.dram_tensor("p2", [128, 6, 128], BF, kind="ExternalInput")
    p3 = nc.dram_tensor("p3", [128, 6, 128], BF, kind="ExternalInput")
    wv = nc.dram_tensor("wv", [128, 6, 256], BF, kind="ExternalInput")
    bcat = nc.dram_tensor("bcat", [128, 3], F32, kind="ExternalInput")
    c2k = nc.dram_tensor("c2k", [64, S], BF, kind="ExternalInput")
    s2k = nc.dram_tensor("s2k", [64, S], BF, kind="ExternalInput")
    maskc = nc.dram_tensor("maskc", [128, NCHUNK], F32, kind="ExternalInput")
    maskab = nc.dram_tensor("maskab", [128, NCHUNK], F32, kind="ExternalInput")
    vones = nc.dram_tensor("vones", [128, 3 * NCHUNK], BF, kind="ExternalInput")
    rowv = nc.dram_tensor("rowv", [1, 384], BF, kind="ExternalInput")
    rowo = nc.dram_tensor("rowo", [1, 64], MDT, kind="ExternalInput")
    out = nc.dram_tensor("out", [HG, 64, SQ], F32, kind="ExternalOutput")

    ADD = mybir.AluOpType.add
    MULT = mybir.AluOpType.mult

    with tile.TileContext(nc) as tc:
        with (
            tc.tile_pool(name="persist", bufs=1) as persist,
            tc.tile_pool(name="small", bufs=1) as small,
        ):
            # persistent per-head tensors; partitions 64:128 of kt/qt hold a
            # duplicate of 0:64 for the 64x128 row-tiled score matmuls
            kt_all = persist.tile([128, HG, S], BF, name="kt", tag="kt")
            qt_all = persist.tile([128, HG, SQ], BF, name="qt", tag="qt")
            vt = persist.tile([128, NCHUNK, HG, 65], BF, name="vt", tag="vt")
            masks = small.tile([128, NCHUNK], F32)
            maskabs = small.tile([128, NCHUNK], F32)
            rv = small.tile([1, 384], BF)
            ro = small.tile([1, 64], MDT)
            scr1 = small.tile([1, 1], F32)
            nc.sync.dma_start(out=rv, in_=rowv[:, :])
            nc.sync.dma_start(out=ro, in_=rowo[:, :])
            onesv = rv[0:1, 0:128]
            bvrt = rv[0:1, 128:384]
            # dummy exp: pulls the ACT exp table load off the critical path
            nc.scalar.activation(scr1, ro[0:1, 0:1], mybir.ActivationFunctionType.Exp)

            IDEN = mybir.ActivationFunctionType.Identity

            # ---------------- projection phase ----------------
            with (
                tc.tile_pool(name="wpool", bufs=1) as wpool,
                tc.tile_pool(name="tabs", bufs=1) as tabs,
                tc.tile_pool(name="hst", bufs=2) as hstp,
                tc.tile_pool(name="pskq", bufs=3, space="PSUM") as pskq,
                tc.tile_pool(name="psv", bufs=2, space="PSUM") as psvp,
                tc.tile_pool(name="prer", bufs=3) as prer,
                tc.tile_pool(name="ropetmp", bufs=4) as rtmp,
            ):
                p1s = wpool.tile([128, 6, 128], BF)
                p2s = wpool.tile([128, 6, 128], BF)
                p3s = wpool.tile([128, 6, 128], BF)
                wvs = wpool.tile([128, 6, 256], BF)
                blo = wpool.tile([64, 3], F32)
                bhi = wpool.tile([64, 3], F32)
                nc.scalar.dma_start(out=p1s, in_=p1[:, :, :])
                nc.sync.dma_start(out=blo, in_=bcat[0:64, :])
                nc.sync.dma_start(out=bhi, in_=bcat[64:128, :])
                c2ks = tabs.tile([64, S], BF)
                s2ks = tabs.tile([64, S], BF)
                nc.scalar.dma_start(out=c2ks, in_=c2k[:, :])
                nc.scalar.dma_start(out=s2ks, in_=s2k[:, :])

                TSUB = nc.vector.tensor_sub
                TADD = nc.vector.tensor_add

                def proj(pw, hst, blo_c, bhi_c, preA_sl, preB_sl):
                    """6-chunk matmul + biased evacuation into [64,*] halves."""
                    ps = pskq.tile([128, 512], F32, name="ps", tag="ps")
                    for ch in range(6):
                        nc.tensor.matmul(
                            ps, pw[:, ch, :], hst[:, ch, :],
                            start=(ch == 0), stop=(ch == 5),
                        )
                    nc.scalar.activation(preA_sl, ps[0:64, :], IDEN, bias=blo_c)
                    if preB_sl is not None:
                        nc.scalar.activation(preB_sl, ps[64:128, :], IDEN, bias=bhi_c)

                def rope_pair(preA, preB, sl, d0, d1, dst, eng):
                    """preA=[h0x1 h1x1], preB=[h0x2 h1x2] -> heads d0, d1 of dst."""
                    m1 = rtmp.tile([64, 1024], BF, name="m1", tag="m1")
                    m2 = rtmp.tile([64, 1024], BF, name="m2", tag="m2")
                    eng.tensor_mul(m1, preA, c2ks[:, sl])
                    eng.tensor_mul(m2, preB, s2ks[:, sl])
                    TSUB(dst[0:32, d0, sl], m1[0:32, :], m2[0:32, :])
                    TSUB(dst[0:32, d1, sl], m1[32:64, :], m2[32:64, :])
                    m3 = rtmp.tile([64, 1024], BF, name="m3", tag="m3")
                    m4 = rtmp.tile([64, 1024], BF, name="m4", tag="m4")
                    eng.tensor_mul(m3, preA, s2ks[:, sl])
                    eng.tensor_mul(m4, preB, c2ks[:, sl])
                    TADD(dst[32:64, d0, sl], m3[0:32, :], m4[0:32, :])
                    TADD(dst[32:64, d1, sl], m3[32:64, :], m4[32:64, :])

                def rope_one(pre, sl, d, dst, eng):
                    """pre [64,1024] = [x1; x2] of one head -> head d of dst."""
                    m1 = rtmp.tile([32, 1024], BF, name="n1", tag="m1")
                    m2 = rtmp.tile([32, 1024], BF, name="n2", tag="m2")
                    eng.tensor_mul(m1, pre[0:32, :], c2ks[0:32, sl])
                    eng.tensor_mul(m2, pre[32:64, :], s2ks[32:64, sl])
                    TSUB(dst[0:32, d, sl], m1, m2)
                    m3 = rtmp.tile([32, 1024], BF, name="n3", tag="m3")
                    m4 = rtmp.tile([32, 1024], BF, name="n4", tag="m4")
                    eng.tensor_mul(m3, pre[0:32, :], s2ks[0:32, sl])
                    eng.tensor_mul(m4, pre[32:64, :], c2ks[32:64, sl])
                    TADD(dst[32:64, d, sl], m3, m4)

                for sp2 in range(4):
                    st0 = 2 * sp2
                    sl2 = bass.ds(sp2 * 1024, 1024)
                    hsts = []
                    for i, st in enumerate((st0, st0 + 1)):
                        hst = hstp.tile([128, 6, 512], BF)
                        if st == 0:
                            for ch in range(6):
                                nc.sync.dma_start(
                                    out=hst[:, ch], in_=hst8[st, :, ch])
                        else:
                            nc.sync.dma_start(out=hst, in_=hst8[st])
                        hsts.append(hst)
                    if sp2 == 0:
                        for t, d in ((p2s, p2), (p3s, p3)):
                            nc.scalar.dma_start(out=t, in_=d[:, :, :])
                        nc.scalar.dma_start(out=wvs, in_=wv[:, :, :])
                    if sp2 == 1:
                        nc.scalar.dma_start(
                            out=vt[:, :, :, 64],
                            in_=vones.rearrange("p (c h) -> p c h", h=HG))
                        nc.scalar.dma_start(out=masks, in_=maskc[:, :])
                        nc.scalar.dma_start(out=maskabs, in_=maskab[:, :])

                    pre1A = prer.tile([64, 1024], BF, name="p1A", tag="pre")
                    pre1B = prer.tile([64, 1024], BF, name="p1B", tag="pre")
                    pre2A = prer.tile([64, 1024], BF, name="p2A", tag="pre")
                    pre2B = prer.tile([64, 1024], BF, name="p2B", tag="pre")
                    for i in range(2):
                        hsl = bass.ds(i * 512, 512)
                        proj(p1s, hsts[i], blo[:, 0:1], bhi[:, 0:1],
                             pre1A[:, hsl], pre1B[:, hsl])
                        proj(p2s, hsts[i], blo[:, 1:2], bhi[:, 1:2],
                             pre2A[:, hsl],
                             pre2B[:, hsl] if sp2 < 2 else None)
                    rope_pair(pre1A, pre1B, sl2, 0, 1, kt_all, nc.gpsimd)
                    rope_one(pre2A, sl2, 2, kt_all, nc.gpsimd)
                    if sp2 < 2:
                        pre3A = prer.tile([64, 1024], BF, name="p3A", tag="pre")
                        pre3B = prer.tile([64, 1024], BF, name="p3B", tag="pre")
                        for i in range(2):
                            hsl = bass.ds(i * 512, 512)
                            proj(p3s, hsts[i], blo[:, 2:3], bhi[:, 2:3],
                                 pre3A[:, hsl], pre3B[:, hsl])
                        rope_one(pre2B, sl2, 2, qt_all, nc.vector)
                        rope_pair(pre3A, pre3B, sl2, 0, 1, qt_all, nc.vector)
                        nc.vector.tensor_copy(
                            qt_all[64:128, :, sl2], qt_all[0:64, :, sl2])
                    nc.gpsimd.tensor_copy(
                        kt_all[64:128, :, sl2], kt_all[0:64, :, sl2])
                    # v projection; bias via K=1 matmul, evacuation on ACT
                    for i in range(2):
                        for sc in range(4):
                            psv = psvp.tile([128, 256], F32, name="psv", tag="psv")
                            for ch in range(6):
                                nc.tensor.matmul(
                                    psv,
                                    hsts[i][:, ch, sc * 128 : (sc + 1) * 128],
                                    wvs[:, ch, :],
                                    start=(ch == 0), stop=False,
                                )
                            nc.tensor.matmul(psv, onesv, bvrt, start=False, stop=True)
                            ci = (st0 + i) * 4 + sc
                            nc.scalar.copy(
                                vt[:, ci, :, 0:64],
                                psv[:, 0:192].rearrange("p (h d) -> p h d", h=HG),
                            )

            # ---------------- attention phase ----------------
            groups = [tuple(range(t, min(t + 3, NCHUNK))) for t in range(0, NCHUNK, 3)]
            with (
                tc.tile_pool(name="scps", bufs=3, space="PSUM") as scps,
                tc.tile_pool(name="ctxps", bufs=1, space="PSUM") as ctxps,
                tc.tile_pool(name="probs", bufs=6) as probsp,
                tc.tile_pool(name="normp", bufs=2) as normp,
                tc.tile_pool(name="outp", bufs=2) as outp,
            ):
                for h in range(HG):
                    for u in range(2):
                        qsl0 = u * 1024
                        ctxp = ctxps.tile([65, 1024], F32, name="ctx", tag="ctx")

                        def flush(pend):
                            for pt, c in pend:
                                for j in range(2):
                                    nc.tensor.matmul(
                                        ctxp[:, j * 512 : (j + 1) * 512],
                                        vt[:, c, h, :],
                                        pt[:, j * 512 : (j + 1) * 512],
                                        start=(c == 0), stop=(c == NCHUNK - 1),
                                    )

                        pend = None
                        for grp in groups:
                            cur = []
                            for c in grp:
                                c0 = c * 128
                                sp = scps.tile([128, 1024], F32, name="sp", tag="sp")
                                nc.tensor.matmul(
                                    sp[:, 0:512],
                                    kt_all[0:64, h, c0 : c0 + 128],
                                    qt_all[0:64, h, qsl0 : qsl0 + 512],
                                    start=True, stop=True,
                                )
                                nc.tensor.matmul(
                                    sp[:, 512:1024],
                                    kt_all[64:128, h, c0 : c0 + 128],
                                    qt_all[64:128, h, qsl0 + 512 : qsl0 + 1024],
                                    start=True, stop=True,
                                )
                                pt = probsp.tile([128, 1024], BF, name="pt", tag="pt")
                                if c % SCHRA_MOD == SCHRA_PHASE:
                                    # fast exp: bf16 bit pattern of A*s + B
                                    nc.vector.tensor_scalar(
                                        pt[:, :].bitcast(I16), sp,
                                        SCHRA_A, maskabs[:, c : c + 1],
                                        MULT, ADD,
                                    )
                                else:
                                    nc.scalar.activation(
                                        pt, sp, mybir.ActivationFunctionType.Exp,
                                        bias=masks[:, c : c + 1],
                                    )
                                cur.append((pt, c))
                            if pend is not None:
                                flush(pend)
                            pend = cur
                        flush(pend)
                        # normalize: broadcast the denominator row via a K=1
                        # matmul, then approx-reciprocal + multiply
                        cs = normp.tile([64, 1024], MDT, name="cs", tag="cs")
                        nc.vector.tensor_copy(cs, ctxp[0:64, :])
                        csd = normp.tile([1, 1024], MDT, name="csd", tag="csd")
                        nc.vector.tensor_copy(csd, ctxp[64:65, :])
                        bc = ctxps.tile([64, 1024], F32, name="bc", tag="ctx")
                        for j in range(2):
                            nc.tensor.matmul(
                                bc[:, j * 512 : (j + 1) * 512],
                                ro[:, :],
                                csd[:, j * 512 : (j + 1) * 512],
                                start=True, stop=True,
                            )
                        rb = normp.tile([64, 1024], F32, name="rb", tag="rb")
                        nc.vector.reciprocal_approx_fast(rb[:, :], bc[:, :])
                        ot = outp.tile([64, 1024], F32, name="ot", tag="ot")
                        nc.gpsimd.tensor_mul(
                            ot, cs[:, :].bitcast(F32), rb)
                        nc.sync.dma_start(
                            out=out[h][:, qsl0 : qsl0 + 1024], in_=ot)

    nc.compile()
    return nc


_NC_CACHE = None


def _get_nc():
    global _NC_CACHE
    if _NC_CACHE is None:
        _NC_CACHE = _build_kernel()
    return _NC_CACHE


def _rope_tables():
    """Bit-identical to the reference's f32 jax-on-cpu tables."""
    import jax
    import jax.numpy as jnp

    cpu = jax.devices("cpu")[0]
    with jax.default_device(cpu):
        inv_freq = 1.0 / (
            10000.0 ** (jnp.arange(0, HD, 2, dtype=jnp.float32) / HD)
        )
        t = jnp.arange(S, dtype=jnp.float32)
        freqs = t[:, None] * inv_freq[None, :]
        cos = np.asarray(jnp.cos(freqs), dtype=np.float32)
        sin = np.asarray(jnp.sin(freqs), dtype=np.float32)
    return cos, sin  # [S, HD2]


def _prep_inputs(hidden_states, attention_mask, Wq, bq, Wk, bk, Wv, bv):
    f = np.float32
    bf = ml_dtypes.bfloat16
    hs = np.asarray(hidden_states, dtype=f).reshape(S, HID)
    mask = np.asarray(attention_mask, dtype=f).reshape(S)
    Wq = np.asarray(Wq, dtype=f)
    Wk = np.asarray(Wk, dtype=f)
    Wv = np.asarray(Wv, dtype=f)
    bq = np.asarray(bq, dtype=f).reshape(HID)
    bk = np.asarray(bk, dtype=f).reshape(HID)
    bv = np.asarray(bv, dtype=f).reshape(HID)

    hsTb = np.ascontiguousarray(hs.T).astype(bf)  # [HID, S]
    scale = f(1.0 / np.sqrt(HD).astype(f))
    WqT = np.ascontiguousarray(Wq.T) * scale  # fold 1/sqrt(d)
    bqs = bq * scale
    WkT = np.ascontiguousarray(Wk.T)
    WvT = np.ascontiguousarray(Wv.T)

    cos, sin = _rope_tables()
    cosT = np.ascontiguousarray(cos.T)  # [32, S]
    sinT = np.ascontiguousarray(sin.T)

    def packed_pair(WT, bvec, i0, i1):
        P = np.concatenate(
            [WT[:, i0 : i0 + 32], WT[:, i1 : i1 + 32],
             WT[:, i0 + 32 : i0 + 64], WT[:, i1 + 32 : i1 + 64]], axis=1)
        b = np.concatenate(
            [bvec[i0 : i0 + 32], bvec[i1 : i1 + 32],
             bvec[i0 + 32 : i0 + 64], bvec[i1 + 32 : i1 + 64]])
        return np.ascontiguousarray(P), np.ascontiguousarray(b.reshape(128, 1))

    def wtile(W):
        # [HID, M] -> [128, 6, M], bf16
        return np.ascontiguousarray(
            W.reshape(6, 128, -1).transpose(1, 0, 2).astype(bf))

    # per-query-half shared tensors (only 2 variants, reused by 4 cores each)
    half = {}
    for hf in range(2):
        perm = np.concatenate([
            np.arange(hf * SQ, hf * SQ + SQ),
            np.arange((1 - hf) * SQ, (1 - hf) * SQ + SQ)])
        cperm = cosT[:, perm]
        sperm = sinT[:, perm]
        c2kv = np.ascontiguousarray(
            np.concatenate([cperm, cperm], axis=0).astype(bf))
        s2kv = np.ascontiguousarray(
            np.concatenate([sperm, sperm], axis=0).astype(bf))
        mperm = mask[perm]
        maskv = np.ascontiguousarray(mperm.reshape(NCHUNK, 128).T)
        maskabv = np.ascontiguousarray(
            (mperm * np.float32(SCHRA_A) + np.float32(SCHRA_B))
            .astype(f).reshape(NCHUNK, 128).T)
        hst8 = np.ascontiguousarray(
            hsTb[:, perm].reshape(6, 128, NST, 512).transpose(2, 1, 0, 3))
        half[hf] = (hst8, c2kv, s2kv, maskv, maskabv)

    in_maps = []
    for core in range(8):
        g, hf = core // 2, core % 2
        i0, i1 = (3 * g) * 64, (3 * g + 1) * 64
        i2 = (3 * g + 2) * 64

        P1, _b1 = packed_pair(WkT, bk, i0, i1)
        P3, _b3 = packed_pair(WqT, bqs, i0, i1)
        b1v = _b1
        b3v = _b3
        P2 = np.ascontiguousarray(
            np.concatenate([WkT[:, i2 : i2 + 64], WqT[:, i2 : i2 + 64]], axis=1))
        b2v = np.ascontiguousarray(
            np.concatenate([bk[i2 : i2 + 64], bqs[i2 : i2 + 64]]).reshape(128, 1))
        bcatv = np.ascontiguousarray(np.concatenate([b1v, b2v, b3v], axis=1))
        wvp = np.zeros((HID, 256), dtype=f)
        wvp[:, :192] = WvT[:, 3 * g * 64 : 3 * g * 64 + 192]
        bvr = np.zeros((1, 256), dtype=f)
        bvr[0, :192] = bv[3 * g * 64 : 3 * g * 64 + 192]
        rowvv = np.ascontiguousarray(
            np.concatenate([np.ones((1, 128), dtype=f), bvr], axis=1).astype(bf))

        hst8, c2kv, s2kv, maskv, maskabv = half[hf]
        in_maps.append({
            "hst8": hst8,
            "p1": wtile(P1), "p2": wtile(P2), "p3": wtile(P3),
            "wv": wtile(wvp),
            "bcat": bcatv,
            "c2k": c2kv, "s2k": s2kv,
            "maskc": maskv, "maskab": maskabv,
            "vones": np.ones((128, 3 * NCHUNK), dtype=ml_dtypes.bfloat16),
            "rowv": rowvv,
            "rowo": np.ones((1, 64), dtype=f),
        })
    return in_maps


def _assemble(results):
    A = np.stack([results[c]["out"] for c in range(8)])  # [8, 3, 64, SQ]
    A = A.reshape(4, 2, HG, 64, SQ)          # [g, hf, j, d, qq]
    full = A.transpose(1, 4, 0, 2, 3).reshape(S, HID)  # [(hf qq), (g j d)]
    return np.ascontiguousarray(full.reshape(1, S, HID).astype(np.float32))


def kernel(hidden_states, attention_mask, Wq, bq, Wk, bk, Wv, bv, _trace=False):
    nc = _get_nc()
    in_maps = _prep_inputs(hidden_states, attention_mask, Wq, bq, Wk, bk, Wv, bv)
    res = run_bass_kernel_spmd(nc, in_maps, core_ids=list(range(8)), trace=_trace)
    out = _assemble(res.results)
    if _trace:
        return out, res
    return out


if __name__ == "__main__":
    rng = np.random.default_rng(0)
    ins = {
        "hidden_states": rng.standard_normal((1, S, HID), dtype=np.float32),
        "attention_mask": np.zeros((1, 1, 1, S), dtype=np.float32),
        "Wq": (rng.standard_normal((HID, HID)) * 0.02).astype(np.float32),
        "bq": np.zeros(HID, np.float32),
        "Wk": (rng.standard_normal((HID, HID)) * 0.02).astype(np.float32),
        "bk": np.zeros(HID, np.float32),
        "Wv": (rng.standard_normal((HID, HID)) * 0.02).astype(np.float32),
        "bv": np.zeros(HID, np.float32),
    }
    out = kernel(**ins)
    print("kernel output", out.shape, out.dtype, np.abs(out).max())
